# revision 47
# baseline (speedup 1.0000x reference)
"""Causal self-attention on 8 trn2 NeuronCores.

Sharding: tensor-parallel over heads (2 heads/core) for QKV+attention.  The
output projection is computed per-core in head-space (each core multiplies its
128 attention dims by its 128-row slice of Wo^T, giving a partial [rows, D]
sum), and four pipelined ReduceScatters (one per 1024-row "quarter", fired as
soon as that quarter's attention finishes) add the partials across cores and
scatter 128 rows back to each core.  This overlaps the collectives with the
remaining attention compute instead of idling 40+us in one big AllToAll.

Schedule: QKV projections run column-group-major (one 512-row group of q/k/v
at a time) and are interleaved with the attention quarters — groups 0,1 up
front, then one group pair between quarters, exactly when that pair's rows are
first needed.  This starts the (activation-engine-bound) attention phase ~15us
in and keeps the PE busy through each quarter's normalization boundary.

All matmuls run in bf16 with f32 PSUM accumulation.  Attention is computed in
"scores transposed" layout ([keys, queries] on chip) so no on-chip transposes
of the probability matrix are needed; softmax denominators come from a ones
column appended to V, and the causal mask is a multiplicative {0,1} bf16 mask
applied after exp (safe: scores are O(6), no overflow without max-subtraction).

Softmax normalization: the first strip of each quarter normalizes via a
reciprocal -> DRAM bounce -> partition-broadcast DMA (latency hidden behind
the second strip's key blocks); the second strip, which is on the critical
path to the quarter's ReduceScatter, broadcasts the reciprocals with a
[1,64]-ones matmul into PSUM plus one copy — no DRAM round trip.

The output bias bo is added on the host (exact f32); everything else stays on
device.
"""

import numpy as np
import ml_dtypes

B, T, D, H, HD = 2, 2048, 1024, 16, 64
NCORES = 8
R = B * T              # 4096 global rows (b*T + t)
HPC = H // NCORES      # 2 heads per core
HDIM = HPC * HD        # 128 dims per core
ROWS_PER_CORE = R // NCORES  # 512
NKT = D // 128         # 8 contraction tiles
NSTRIP = T // 512      # 4 query strips per batch
NQ = 4                 # row quarters (1024 rows each), one ReduceScatter per

_BF16 = ml_dtypes.bfloat16
_cache = {}


def _patch_tile_drain():
    """This walrus build rejects >1 sync wait on SP CTRL instructions; split
    the Tile tail-drain waits across single-wait nops."""
    import concourse.mybir as mybir
    import concourse.tile as tile_mod
    from concourse.vector_clock import ScopedClock

    if getattr(tile_mod.TileContext, "_drain_patched", False):
        return

    def _drain_and_barrier(self, tick_clock, wait_clock):
        nc = self.nc
        dummy = mybir.InstNoOp(
            name=nc.get_next_instruction_name(),
            engine=mybir.EngineType.SP,
            ins=[],
            outs=[],
        )
        wait_clock.add_sem_waits(dummy, ScopedClock({None: tick_clock.global_clock}))
        waits = list(dummy.sync_info.on_wait) if dummy.sync_info else []
        for i in range(len(waits)):
            w = nc.sync.nop(nofuse=True, hint="tail_drain_wait")
            w.ins.sync_info = mybir.SyncInfo(on_wait=waits[i : i + 1], on_update=[])
        nc.sync.drain()
        nc.all_engine_barrier()
        assert self.sems is not None
        popped = nc._tile_sem_poison_stack.pop()
        assert popped is self._sem_poison
        nc.clear_and_free_semaphores(list(self.sems.allocated().values()))
        nc.all_engine_barrier()

    tile_mod.TileContext._drain_and_barrier = _drain_and_barrier

    # Body instructions can also accumulate >2 waits (CTRL structs take 1,
    # other structs 2 on this walrus).  Before lowering, move excess waits
    # onto single-wait nops inserted just before the instruction on the same
    # engine stream.
    _orig_lower = tile_mod.TileContext._lower_ordered_insts

    def _lower_split_waits(self, ordered):
        nc = self.nc
        for bb_name, insts in ordered.items():
            new_insts = []
            for inst in insts:
                si = getattr(inst, "sync_info", None)
                waits = list(si.on_wait) if si is not None and si.on_wait else []
                limit = 1
                if len(waits) > limit and inst.engine is not None:
                    keep = waits[: limit - 1] if limit > 1 else []
                    spill = waits[len(keep) :][:-1]
                    keep = keep + [waits[-1]]
                    for w in spill:
                        nop = mybir.InstNoOp(
                            name=nc.get_next_instruction_name(),
                            engine=inst.engine,
                            ins=[],
                            outs=[],
                        )
                        nop.sync_info = mybir.SyncInfo(on_wait=[w], on_update=[])
                        nop.debug = inst.debug
                        new_insts.append(nop)
                    inst.sync_info = mybir.SyncInfo(
                        on_wait=keep, on_update=list(si.on_update or [])
                    )
                new_insts.append(inst)
            ordered[bb_name] = new_insts
        return _orig_lower(self, ordered)

    tile_mod.TileContext._lower_ordered_insts = _lower_split_waits
    tile_mod.TileContext._drain_patched = True


def _build():
    import concourse.bass as bass
    import concourse.mybir as mybir
    import concourse.tile as tile
    from concourse.tile import add_dep_helper
    from concourse.masks import make_identity
    import concourse.bass as _bass

    _patch_tile_drain()
    f32 = mybir.dt.float32
    bf16 = mybir.dt.bfloat16

    nc = bass.Bass("TRN2", target_bir_lowering=False, debug=False, num_devices=NCORES)

    # ---- DRAM I/O (per core) ----
    xT = nc.dram_tensor("xT", [D, R], bf16, kind="ExternalInput").ap()
    # host-pretiled: [p, k*HDIM + c] = W.T[(k*128 + p), c] for this core's slice
    wqT = nc.dram_tensor("wqT", [128, NKT * HDIM], bf16, kind="ExternalInput").ap()
    wkT = nc.dram_tensor("wkT", [128, NKT * HDIM], bf16, kind="ExternalInput").ap()
    wvT = nc.dram_tensor("wvT", [128, NKT * HDIM], bf16, kind="ExternalInput").ap()
    bq_s = nc.dram_tensor("bq_s", [HDIM, 1], f32, kind="ExternalInput").ap()
    bk_s = nc.dram_tensor("bk_s", [HDIM, 1], f32, kind="ExternalInput").ap()
    bv_s = nc.dram_tensor("bv_s", [HDIM, 1], f32, kind="ExternalInput").ap()
    # core's 128-row slice of Wo^T: rows = this core's attention dims
    woT_c = nc.dram_tensor("woT_c", [HDIM, D], bf16, kind="ExternalInput").ap()
    masks_d = nc.dram_tensor("masks", [128, 4 * 1024], bf16, kind="ExternalInput").ap()
    # per-core output: quarter q rows land at [128q, 128(q+1)); host reassembles
    out = nc.dram_tensor("out", [NQ * 128, D], bf16, kind="ExternalOutput").ap()

    # ReduceScatter buffers per quarter: partial rows (quarter-local) in, 128
    # summed rows out
    cc_in = [nc.dram_tensor(f"cc_in{q}", [1024, D], bf16).ap() for q in range(NQ)]
    rs_out = [nc.dram_tensor(f"rs_out{q}", [128, D], bf16).ap() for q in range(NQ)]
    # first-strip softmax reciprocals bounce: rows 2j (head A), 2j+1 (head B)
    rec_d = nc.dram_tensor("rec_d", [2 * B * NSTRIP, 512], bf16).ap()

    wq = {"v": wvT, "q": wqT, "k": wkT}
    wb = {"v": bv_s, "q": bq_s, "k": bk_s}
    scale = 1.0 / float(np.sqrt(HD))

    with tile.TileContext(nc) as tc:
        import contextlib

        with contextlib.ExitStack() as ctx:
            singles = ctx.enter_context(tc.tile_pool(name="singles", bufs=1))

            # ---- on-chip constants first (Pool/DVE/Act engine work at t=0,
            # before any SWDGE descriptor generation queues up on Pool) ----
            ident = singles.tile([128, 128], bf16, tag="ident")
            make_identity(nc, ident)
            ones_row = singles.tile([1, 128], bf16, tag="ones")
            nc.vector.memset(ones_row, 1.0)
            # preload the Exp activation table off the critical path
            exp_warm = singles.tile([1, 128], bf16, tag="expwarm")
            nc.scalar.activation(
                out=exp_warm,
                in_=ones_row,
                func=mybir.ActivationFunctionType.Exp,
                scale=1.0,
            )

            # ---- weight/x DMAs.  SP queue (HWDGE): wv, x even-k chunks of
            # groups 0/1, wq, wk, masks, remaining even-k chunks.  Pool queue
            # (SWDGE; descriptor gen runs on the Pool engine, in parallel with
            # HWDGE): odd-k chunks, wo.  Biases ride the idle Act queue.
            # One SBUF tile per x chunk so QKV matmuls wait only on the exact
            # chunk they read.
            w_sb = {}
            bias_sb = {}
            for name in ("v", "q", "k"):
                w_sb[name] = singles.tile(
                    [128, NKT, HDIM], bf16, tag=f"w{name}", name=f"w{name}"
                )
                bias_sb[name] = singles.tile(
                    [HDIM, 1], f32, tag=f"b{name}", name=f"b{name}"
                )

            col_chunks = [(0, 512), (512, 1024), (1024, 2048), (2048, 3072), (3072, 4096)]
            xt_sb = [[None] * len(col_chunks) for _ in range(NKT)]

            def load_x_chunk(ci, k):
                lo, hi = col_chunks[ci]
                t = singles.tile(
                    [128, hi - lo], bf16, tag=f"xt{k}_{ci}", name=f"xt{k}_{ci}"
                )
                [nc.sync, nc.gpsimd][k % 2].dma_start(
                    out=t, in_=xT[128 * k : 128 * (k + 1), lo:hi]
                )
                xt_sb[k][ci] = t

            # weights arrive host-pretiled ([D, HDIM] -> [128, k, c] packed),
            # so each is one contiguous 2KB-per-partition DMA
            nc.sync.dma_start(
                out=w_sb["v"], in_=wq["v"].rearrange("p (k c) -> p k c", c=HDIM)
            )
            for k in range(0, NKT, 2):  # even k, chunk 0 (SP)
                load_x_chunk(0, k)
            for k in range(1, NKT, 2):  # odd k, chunk 0 (Pool)
                load_x_chunk(0, k)
            wo_sb = singles.tile([128, D], bf16, tag="wo")
            nc.gpsimd.dma_start(out=wo_sb, in_=woT_c)
            for name in ("q", "k"):
                nc.sync.dma_start(
                    out=w_sb[name],
                    in_=wq[name].rearrange("p (k c) -> p k c", c=HDIM),
                )
            for k in range(NKT):
                load_x_chunk(1, k)
            # biases ride late: their HWDGE slots must not delay the x chunks
            # that gate the first projections (first bias-add is at ~7us)
            for name in ("v", "q", "k"):
                nc.scalar.dma_start(out=bias_sb[name], in_=wb[name])
            mask_sb = singles.tile([128, 4 * 1024], bf16, tag="mask")
            nc.sync.dma_start(out=mask_sb, in_=masks_d)
            for ci in (2, 3, 4):
                for k in range(NKT):
                    load_x_chunk(ci, k)

            def xchunk(k, g):
                """x tile + column slice for 512-col group g of k-tile k."""
                ci, off = (g, 0) if g < 2 else (2 + (g - 2) // 2, 512 * ((g - 2) % 2))
                return xt_sb[k][ci][:, off : off + 512]

            # persistent activations
            qT_sb = singles.tile([128, R], bf16, tag="qT")   # rows 0-63 head A dims
            kT_sb = singles.tile([128, R], bf16, tag="kT")
            vT_sb = singles.tile([128, R], bf16, tag="vT")
            # v in [keys, dims] layout per key block kb:
            #   cols 0:64 = vA, 64 = ones, 65:129 = vB, 129 = ones
            # so lhsT for head h = cols [65h, 65h+65) = [v_h | ones]:
            # attnout at psum rows 0-63, softmax sums at row 64.
            v_ab = singles.tile([128, R // 128, 130], bf16, tag="vab")
            nc.vector.memset(v_ab[:, :, 64:65], 1.0)
            nc.vector.memset(v_ab[:, :, 129:130], 1.0)
            dests = {"v": vT_sb, "q": qT_sb, "k": kT_sb}

            # ---- PSUM rings (8 banks total):
            #   "sc"  2x[128,512]f32 (2 banks): scores (one tile per strip);
            #         borrowed for the second-strip reciprocal broadcast
            #   "fl"  2x[128,512]f32 (2 banks): woven QKV-group psums and V
            #         transposes (the PE "filler" stream)
            #   "av0" 2x[128,512]f32 (2 banks): strip-s0 psV pair, then s0's
            #         output-projection psums
            #   "av1" 2x[128,512]f32 (2 banks): same for strip s1
            with (
                tc.tile_pool(name="sc_ps", bufs=2, space="PSUM") as scp,
                tc.tile_pool(name="av_ps", bufs=2, space="PSUM") as avp,
                tc.tile_pool(name="p_sb", bufs=18) as ppool,
                tc.tile_pool(name="att_sb", bufs=6) as apool,
                tc.tile_pool(name="bc_sb", bufs=8) as bpool,
                tc.tile_pool(name="rec_sb", bufs=4) as rpool,
                tc.tile_pool(name="op_sb", bufs=8) as opool,
            ):
                av_tags = ["av0", "av1"]

                # QKV "filler" stream: each unit is a small closure (2 matmuls
                # of a projection, a bias add, or one V transpose) pulled
                # between attention key blocks so the PE fills the slack of
                # the exp-bound cadence with projection work.
                filler = []

                def push_qkv_group(g):
                    # q,k first: the next quarter's scores need only those;
                    # v (+transposes) is consumed one kb later by attnV
                    for name in ("q", "k", "v"):
                        holder = {}

                        def mm_pair(j, name=name, holder=holder, g=g):
                            def f():
                                if j == 0:
                                    holder["ps"] = avp.tile(
                                        [128, 512], f32, tag="fl",
                                        name=f"ps_{name}{g}",
                                    )
                                for k in (2 * j, 2 * j + 1):
                                    nc.tensor.matmul(
                                        holder["ps"],
                                        w_sb[name][:, k, :],
                                        xchunk(k, g),
                                        start=(k == 0),
                                        stop=(k == NKT - 1),
                                    )
                                if j == 3:
                                    nc.vector.tensor_scalar_add(
                                        dests[name][:, 512 * g : 512 * (g + 1)],
                                        holder["ps"],
                                        bias_sb[name],
                                    )
                            return f

                        for j in range(4):
                            filler.append(mm_pair(j))
                    for kb in range(4 * g, 4 * (g + 1)):
                        def tr(kb=kb):
                            tps = avp.tile([128, 128], bf16, tag="fl", name=f"vt{kb}")
                            nc.tensor.transpose(
                                tps, vT_sb[:, 128 * kb : 128 * (kb + 1)], ident
                            )
                            # strided copy: halves -> cols [0:64] and [65:129]
                            nc.vector.tensor_copy(
                                v_ab[:, kb, :].rearrange(
                                    "p (gg c) -> p gg c", c=65
                                )[:, :, 0:64],
                                tps.rearrange("p (gg c) -> p gg c", c=64),
                            )
                        filler.append(tr)

                def pull_filler(n):
                    for _ in range(n):
                        if filler:
                            filler.pop(0)()

                def emit_qkv_group_fast(g, tags=("fl", "av0", "av1")):
                    """Upfront group emission using three rings in parallel."""
                    for name, tag in zip(("q", "k", "v"), tags):
                        ps = avp.tile([128, 512], f32, tag=tag, name=f"ps_{name}{g}")
                        for k in range(NKT):
                            nc.tensor.matmul(
                                ps,
                                w_sb[name][:, k, :],
                                xchunk(k, g),
                                start=(k == 0),
                                stop=(k == NKT - 1),
                            )
                        nc.vector.tensor_scalar_add(
                            dests[name][:, 512 * g : 512 * (g + 1)],
                            ps,
                            bias_sb[name],
                        )
                    for kb in range(4 * g, 4 * (g + 1)):
                        tps = avp.tile([128, 128], bf16, tag=tags[0], name=f"vt{kb}")
                        nc.tensor.transpose(
                            tps, vT_sb[:, 128 * kb : 128 * (kb + 1)], ident
                        )
                        nc.vector.tensor_copy(
                            v_ab[:, kb, :].rearrange(
                                "p (gg c) -> p gg c", c=65
                            )[:, :, 0:64],
                            tps.rearrange("p (gg c) -> p gg c", c=64),
                        )

                def emit_outproj(att, quarter, si, cc_writes, engs, fill=0):
                    """Output projection for one 512-row strip (4 blocks of
                    128 rows); att is the normalized [128 dims, 512 rows]
                    lhsT.  Partials go quarter-local into cc_in[quarter].
                    Copy engines must be DVE/ACT (GPSIMD cannot read PSUM)."""
                    cp_engs, wr_engs = engs
                    split_wr = False
                    for r in range(4):
                        cop = opool.tile([128, D], bf16, tag="op")
                        for n in range(2):
                            ps = avp.tile([128, 512], f32, tag=av_tags[si])
                            nc.tensor.matmul(
                                ps,
                                att[:, 128 * r : 128 * (r + 1)],
                                wo_sb[:, 512 * n : 512 * (n + 1)],
                                start=True,
                                stop=True,
                            )
                            eng = cp_engs[(2 * r + n) % len(cp_engs)]
                            if eng is nc.scalar:
                                eng.copy(cop[:, 512 * n : 512 * (n + 1)], ps)
                            else:
                                eng.tensor_copy(cop[:, 512 * n : 512 * (n + 1)], ps)
                        pull_filler(fill)
                        rows = slice(512 * si + 128 * r, 512 * si + 128 * (r + 1))
                        if split_wr:
                            # tail-critical strip: two half-width writes land
                            # in parallel on separate queues
                            for n, eng in enumerate(wr_engs):
                                wr = eng.dma_start(
                                    out=cc_in[quarter][rows, 512 * n : 512 * (n + 1)],
                                    in_=cop[:, 512 * n : 512 * (n + 1)],
                                )
                                cc_writes.append(wr)
                        else:
                            wr = wr_engs[0].dma_start(
                                out=cc_in[quarter][rows, :], in_=cop
                            )
                            cc_writes.append(wr)

                rs_ccs = []

                def emit_quarter(b, s0, boundary_groups):
                    s1 = s0 + 1
                    quarter = 2 * b + s0 // 2
                    j0 = b * NSTRIP + s0
                    last_pair = quarter == NQ - 1
                    qc1 = slice(T * b + 512 * s1, T * b + 512 * (s1 + 1))
                    for g in boundary_groups:
                        push_qkv_group(g)
                    psV = {}
                    for sx in (s0, s1):
                        for h in ("A", "B"):
                            psV[(h, sx)] = avp.tile(
                                [128, 512],
                                f32,
                                tag=av_tags[sx - s0],
                                name=f"psV_{h}{b}{sx}",
                            )
                    nkb0, nkb1 = 4 * (s0 + 1), 4 * (s1 + 1)
                    cc_writes = []

                    def emit_attnv(kb, p_of):
                        # attnV accumulate; lhsT = [v_h | ones]: attnout rows
                        # 0-63, softmax sums row 64.  Runs one kb behind the
                        # scores so the PE never waits on the exp it just fed.
                        gkb = (T // 128) * b + kb
                        both = kb < nkb0
                        m = kb - 4 * (s0 if both else s1)
                        off = 128 * m if m >= 0 else 0
                        for hi, h in enumerate(("A", "B")):
                            p0, p1 = p_of[h]
                            lhsT = v_ab[:, gkb, 65 * hi : 65 * hi + 65]
                            if both:
                                nc.tensor.matmul(
                                    psV[(h, s0)][0:65, off:512],
                                    lhsT,
                                    p0[:, off:512],
                                    start=(kb == 0),
                                    stop=(kb == nkb0 - 1),
                                )
                                nc.tensor.matmul(
                                    psV[(h, s1)][0:65, 0:512],
                                    lhsT,
                                    p1,
                                    start=(kb == 0),
                                    stop=(kb == nkb1 - 1),
                                )
                            else:
                                nc.tensor.matmul(
                                    psV[(h, s1)][0:65, off:512],
                                    lhsT,
                                    p1[:, off:512],
                                    start=False,
                                    stop=(kb == nkb1 - 1),
                                )

                    def emit_norm_s0():
                        # strip s0: reciprocal -> DRAM bounce -> partition-
                        # broadcast DMA -> normalize.  Latency hidden behind
                        # strip s1's remaining key blocks.
                        rec = rpool.tile([128, 1024], bf16, tag="rec")
                        with nc.allow_low_precision("bf16 softmax recip"):
                            nc.vector.reciprocal(
                                rec[64:65, 0:512], psV[("A", s0)][64:65, :]
                            )
                            nc.vector.reciprocal(
                                rec[64:65, 512:1024], psV[("B", s0)][64:65, :]
                            )
                        nc.gpsimd.dma_start(
                            out=rec_d[2 * j0 : 2 * j0 + 2, :], in_=rec[64:65, :]
                        )
                        att = apool.tile([128, 512], bf16, tag="att")
                        for hi, h in enumerate(("A", "B")):
                            r_ap = rec_d[2 * j0 + hi : 2 * j0 + hi + 1, :]
                            bc_ap = _bass.AP(
                                tensor=r_ap.tensor,
                                offset=r_ap.offset,
                                ap=[[0, 64]] + list(r_ap.ap[1:]),
                            )
                            bc = bpool.tile([64, 512], bf16, tag="bc")
                            nc.gpsimd.dma_start(out=bc, in_=bc_ap)
                            nc.vector.tensor_mul(
                                att[64 * hi : 64 * (hi + 1), :],
                                psV[(h, s0)][0:64, :],
                                bc,
                            )
                        return att

                    def emit_norm_s1():
                        # strip s1 is on the critical path to the quarter's
                        # ReduceScatter: broadcast reciprocals via a
                        # [1,64]-ones matmul into a borrowed scores slot plus
                        # one PSUM->SBUF copy — no DRAM round trip.
                        rec = rpool.tile([1, 1024], bf16, tag="rec")
                        with nc.allow_low_precision("bf16 softmax recip"):
                            nc.vector.reciprocal(
                                rec[0:1, 0:512], psV[("A", s1)][64:65, :]
                            )
                            nc.vector.reciprocal(
                                rec[0:1, 512:1024], psV[("B", s1)][64:65, :]
                            )
                        # bcp lives in the av0 ring (free after s0's outproj):
                        # borrowing the scores ring would couple this
                        # quarter's normalize into the next quarter's scores
                        bcp = avp.tile([128, 512], f32, tag="av0", name="bc_ps")
                        nc.tensor.matmul(
                            bcp[0:64, :],
                            ones_row[0:1, 0:64],
                            rec[0:1, 0:512],
                            start=True,
                            stop=True,
                        )
                        nc.tensor.matmul(
                            bcp[64:128, :],
                            ones_row[0:1, 0:64],
                            rec[0:1, 512:1024],
                            start=True,
                            stop=True,
                        )
                        bcs = bpool.tile([128, 512], bf16, tag="bcs", bufs=2)
                        nc.scalar.copy(bcs, bcp)
                        att = apool.tile([128, 512], bf16, tag="att")
                        nc.vector.tensor_mul(
                            att[0:64, :], psV[("A", s1)][0:64, :], bcs[0:64, :]
                        )
                        nc.vector.tensor_mul(
                            att[64:128, :], psV[("B", s1)][0:64, :], bcs[64:128, :]
                        )
                        return att

                    prev = None
                    for kb in range(nkb1):
                        krange = slice(T * b + 128 * kb, T * b + 128 * (kb + 1))
                        both = kb < nkb0
                        p_of = {}
                        # scores + exp + mask for BOTH heads and both strips;
                        # separate [128,512] psum tiles per strip keep the
                        # scores ring at 2 banks so the QKV filler gets PSUM
                        for hi, h in enumerate(("A", "B")):
                            rows = slice(64 * hi, 64 * (hi + 1))
                            p0 = p1 = None
                            if both:
                                m = kb - 4 * s0
                                off = 128 * m if m >= 0 else 0
                                psS0 = scp.tile([128, 512], f32, tag="sc")
                                p0 = ppool.tile([128, 512], bf16, tag="p")
                                nc.tensor.matmul(
                                    psS0[:, off:512],
                                    kT_sb[rows, krange],
                                    qT_sb[
                                        rows,
                                        T * b + 512 * s0 + off
                                        : T * b + 512 * (s0 + 1),
                                    ],
                                    start=True,
                                    stop=True,
                                    tile_position=(64 * hi, 0),
                                )
                                psS1 = scp.tile([128, 512], f32, tag="sc")
                                p1 = ppool.tile([128, 512], bf16, tag="p")
                                nc.tensor.matmul(
                                    psS1,
                                    kT_sb[rows, krange],
                                    qT_sb[rows, qc1],
                                    start=True,
                                    stop=True,
                                    tile_position=(64 * hi, 0),
                                )
                                nc.scalar.activation(
                                    out=p0[:, off:512],
                                    in_=psS0[:, off:512],
                                    func=mybir.ActivationFunctionType.Exp,
                                    scale=scale,
                                )
                                if m >= 0:
                                    # mask the diagonal block: DVE 4x mode
                                    # (~200ns) vs GPSIMD's 0.42-efficiency
                                    # software multiply (~1us)
                                    nc.vector.tensor_mul(
                                        p0[:, off:512],
                                        p0[:, off:512],
                                        mask_sb[:, 1024 * m + off : 1024 * m + 512],
                                    )
                                nc.scalar.activation(
                                    out=p1,
                                    in_=psS1,
                                    func=mybir.ActivationFunctionType.Exp,
                                    scale=scale,
                                )
                            else:
                                m = kb - 4 * s1
                                off = 128 * m if m >= 0 else 0
                                psS1 = scp.tile([128, 512], f32, tag="sc")
                                p1 = ppool.tile([128, 512], bf16, tag="p")
                                nc.tensor.matmul(
                                    psS1[:, off:512],
                                    kT_sb[rows, krange],
                                    qT_sb[
                                        rows,
                                        T * b + 512 * s1 + off
                                        : T * b + 512 * (s1 + 1),
                                    ],
                                    start=True,
                                    stop=True,
                                    tile_position=(64 * hi, 0),
                                )
                                nc.scalar.activation(
                                    out=p1[:, off:512],
                                    in_=psS1[:, off:512],
                                    func=mybir.ActivationFunctionType.Exp,
                                    scale=scale,
                                )
                                if m >= 0:
                                    nc.vector.tensor_mul(
                                        p1[:, off:512],
                                        p1[:, off:512],
                                        mask_sb[:, 1024 * m + off : 1024 * m + 512],
                                    )
                            p_of[h] = (p0, p1)
                            if hi == 0 and prev is not None:
                                # weave the previous block's attnV between the
                                # two heads' scores: covers the exp latency
                                # that gates the scores-ring reuse
                                emit_attnv(*prev)
                                if prev[0] == nkb0 - 1:
                                    # strip s0 complete: normalize + project
                                    # it while s1's key blocks keep PE/ACT
                                    # busy.  Normal priority: hoisting this
                                    # chain would stall the remaining key
                                    # blocks' scores behind it on the
                                    # in-order PE stream.
                                    att0 = emit_norm_s0()
                                    emit_outproj(
                                        att0,
                                        quarter,
                                        0,
                                        cc_writes,
                                        ([nc.vector, nc.scalar]
                                         if last_pair else [nc.vector],
                                         [nc.sync]),
                                    )
                        prev = (kb, p_of)
                        pull_filler(-(-len(filler) // max(1, nkb1 - kb)))
                    emit_attnv(*prev)
                    # boundary: the second strip's normalize + projection +
                    # ReduceScatter; high priority only for the last quarter,
                    # whose chain sets the final ReduceScatter (and thus
                    # program end); earlier quarters have collective-queue
                    # slack and should yield to the next quarter's exp stream
                    with (tc.high_priority() if last_pair
                          else contextlib.nullcontext()):
                        att1 = emit_norm_s1()
                        # ACT is idle once the quarter's exps drain, so it
                        # takes half the boundary copies; DVE takes the rest
                        engs = (
                            ([nc.scalar, nc.vector], [nc.sync, nc.scalar])
                            if last_pair
                            else ([nc.vector, nc.scalar, nc.vector, nc.vector], [nc.sync])
                        )
                        emit_outproj(att1, quarter, 1, cc_writes, engs)
                        # fire this quarter's ReduceScatter (adds partials
                        # across cores; core j gets rows [128j, 128(j+1)))
                        cc = nc.gpsimd.collective_compute(
                            "ReduceScatter",
                            mybir.AluOpType.add,
                            ins=[cc_in[quarter]],
                            outs=[rs_out[quarter]],
                            replica_groups=[list(range(NCORES))],
                        )
                        for wr in cc_writes:
                            add_dep_helper(
                                cc.ins, wr.ins, sync=True, reason="cc in ready"
                            )
                    rs_ccs.append((quarter, cc))
                    # drain any remaining woven QKV work before the next
                    # quarter's scores need its projections
                    pull_filler(len(filler))

                # warm the PE p-state with ~3us of dependency-free matmuls
                # so the QKV prefix runs at full speed from its first cycle
                warm = avp.tile([128, 512], f32, tag="fl", name="warm_ps")
                for i in range(24):
                    nc.tensor.matmul(
                        warm[:, 0:128], ident, ident, start=True, stop=True
                    )
                emit_qkv_group_fast(0)
                push_qkv_group(1)
                emit_quarter(0, 0, (2, 3))      # quarter 0: rows 0-1023
                emit_quarter(0, 2, (4, 5, 6))   # quarter 1: rows 1024-2047
                emit_quarter(1, 0, (7,))        # quarter 2: rows 2048-3071
                emit_quarter(1, 2, ())          # quarter 3: rows 3072-4095

                # final hops: DRAM->DRAM, gated on each quarter's collective.
                # tile_wait_until keeps the greedy scheduler from dispatching
                # these into idle queue slots mid-program, where their
                # collective-semaphore waits would head-of-line-block the
                # queue (observed: a d2d parked on SP stalled a whole quarter)
                with tc.tile_wait_until(1.0):
                    for quarter, cc in rs_ccs:
                        if quarter == NQ - 1:
                            # tail-critical: two parallel half hops
                            for half, eng in ((0, nc.sync), (1, nc.scalar)):
                                lo = 128 * quarter + 64 * half
                                mv = eng.dma_start(
                                    out=out[lo : lo + 64, :],
                                    in_=rs_out[quarter][64 * half : 64 * half + 64, :],
                                )
                                add_dep_helper(
                                    mv.ins, cc.ins, sync=True, reason="rs done"
                                )
                        else:
                            eng = nc.sync if quarter % 2 == 0 else nc.scalar
                            mv = eng.dma_start(
                                out=out[128 * quarter : 128 * (quarter + 1), :],
                                in_=rs_out[quarter],
                            )
                            add_dep_helper(
                                mv.ins, cc.ins, sync=True, reason="rs done"
                            )

    return nc


def _host_prep(x, Wq, bq, Wk, bk, Wv, bv, Wo, bo):
    """Build the 8 per-core input maps."""
    x = np.asarray(x, np.float32)
    xT = np.ascontiguousarray(x.reshape(R, D).T).astype(_BF16)
    woT = np.ascontiguousarray(np.asarray(Wo, np.float32).T).astype(_BF16)

    # multiplicative causal masks for the 4 diagonal offsets; each 1024-col
    # block m is [mask_m | ones] so a merged strip-pair tile can be masked in
    # one op (second strip fully visible when the first is on the diagonal).
    masks = np.ones((128, 4 * 1024), np.float32)
    r = np.arange(128)[:, None]
    c = np.arange(512)[None, :]
    for m in range(4):
        masks[:, 1024 * m : 1024 * m + 512] = (r + 128 * m <= c).astype(np.float32)
    masks = masks.astype(_BF16)

    def tile_w(W, hs):
        # [D, HDIM] -> [128, NKT*HDIM] with [p, k*HDIM+c] = W.T[k*128+p, c]
        wT = np.asarray(W, np.float32)[hs, :].T.astype(_BF16)
        return np.ascontiguousarray(
            wT.reshape(NKT, 128, HDIM).transpose(1, 0, 2).reshape(128, NKT * HDIM)
        )

    in_maps = []
    for core in range(NCORES):
        hs = slice(HDIM * core, HDIM * (core + 1))
        in_maps.append(
            {
                "xT": xT,
                "wqT": tile_w(Wq, hs),
                "wkT": tile_w(Wk, hs),
                "wvT": tile_w(Wv, hs),
                "bq_s": np.asarray(bq, np.float32)[hs].reshape(HDIM, 1).copy(),
                "bk_s": np.asarray(bk, np.float32)[hs].reshape(HDIM, 1).copy(),
                "bv_s": np.asarray(bv, np.float32)[hs].reshape(HDIM, 1).copy(),
                "woT_c": np.ascontiguousarray(woT[hs, :]),
                "masks": masks,
            }
        )
    return in_maps


def _run(in_maps, trace=False):
    from concourse import bass_utils

    if "nc" not in _cache:
        _cache["nc"] = _build()
    nc = _cache["nc"]
    if trace:
        try:
            res = bass_utils.run_bass_kernel_spmd(
                nc, in_maps, core_ids=list(range(NCORES)), trace=True
            )
            return res
        except Exception:
            pass  # NTFF hook unavailable under this axon build
    try:
        res = bass_utils.run_bass_kernel_spmd(
            nc, in_maps, core_ids=list(range(NCORES)), trace=False
        )
    except Exception:
        # transient device faults (NRT_EXEC_UNIT_UNRECOVERABLE) clear on retry
        res = bass_utils.run_bass_kernel_spmd(
            nc, in_maps, core_ids=list(range(NCORES)), trace=False
        )
    return res


def kernel(x, Wq, bq, Wk, bk, Wv, bv, Wo, bo, _trace=False, _want_results=False):
    in_maps = _host_prep(x, Wq, bq, Wk, bk, Wv, bv, Wo, bo)
    res = _run(in_maps, trace=_trace)
    # core j's output rows: quarter q chunk = global rows 1024q + [128j, 128(j+1))
    full = np.zeros((R, D), np.float32)
    for j in range(NCORES):
        part = np.asarray(res.results[j]["out"], np.float32)
        for q in range(NQ):
            full[1024 * q + 128 * j : 1024 * q + 128 * (j + 1)] = part[
                128 * q : 128 * (q + 1)
            ]
    full += np.asarray(bo, np.float32)[None, :]
    full = full.reshape(B, T, D)
    if _want_results:
        return full, res
    return full


# revision 48
# speedup vs baseline: 1.0021x; 1.0021x over previous
"""Causal self-attention on 8 trn2 NeuronCores.

Sharding: tensor-parallel over heads (2 heads/core) for QKV+attention.  The
output projection is computed per-core in head-space (each core multiplies its
128 attention dims by its 128-row slice of Wo^T, giving a partial [rows, D]
sum), and four pipelined ReduceScatters (one per 1024-row "quarter", fired as
soon as that quarter's attention finishes) add the partials across cores and
scatter 128 rows back to each core.  This overlaps the collectives with the
remaining attention compute instead of idling 40+us in one big AllToAll.

Schedule: QKV projections run column-group-major (one 512-row group of q/k/v
at a time) and are interleaved with the attention quarters — groups 0,1 up
front, then one group pair between quarters, exactly when that pair's rows are
first needed.  This starts the (activation-engine-bound) attention phase ~15us
in and keeps the PE busy through each quarter's normalization boundary.

All matmuls run in bf16 with f32 PSUM accumulation.  Attention is computed in
"scores transposed" layout ([keys, queries] on chip) so no on-chip transposes
of the probability matrix are needed; softmax denominators come from a ones
column appended to V, and the causal mask is a multiplicative {0,1} bf16 mask
applied after exp (safe: scores are O(6), no overflow without max-subtraction).

Softmax normalization: the first strip of each quarter normalizes via a
reciprocal -> DRAM bounce -> partition-broadcast DMA (latency hidden behind
the second strip's key blocks); the second strip, which is on the critical
path to the quarter's ReduceScatter, broadcasts the reciprocals with a
[1,64]-ones matmul into PSUM plus one copy — no DRAM round trip.

The output bias bo is added on the host (exact f32); everything else stays on
device.
"""

import numpy as np
import ml_dtypes

B, T, D, H, HD = 2, 2048, 1024, 16, 64
NCORES = 8
R = B * T              # 4096 global rows (b*T + t)
HPC = H // NCORES      # 2 heads per core
HDIM = HPC * HD        # 128 dims per core
ROWS_PER_CORE = R // NCORES  # 512
NKT = D // 128         # 8 contraction tiles
NSTRIP = T // 512      # 4 query strips per batch
NQ = 4                 # row quarters (1024 rows each), one ReduceScatter per

_BF16 = ml_dtypes.bfloat16
_cache = {}


def _patch_tile_drain():
    """This walrus build rejects >1 sync wait on SP CTRL instructions; split
    the Tile tail-drain waits across single-wait nops."""
    import concourse.mybir as mybir
    import concourse.tile as tile_mod
    from concourse.vector_clock import ScopedClock

    if getattr(tile_mod.TileContext, "_drain_patched", False):
        return

    def _drain_and_barrier(self, tick_clock, wait_clock):
        nc = self.nc
        dummy = mybir.InstNoOp(
            name=nc.get_next_instruction_name(),
            engine=mybir.EngineType.SP,
            ins=[],
            outs=[],
        )
        wait_clock.add_sem_waits(dummy, ScopedClock({None: tick_clock.global_clock}))
        waits = list(dummy.sync_info.on_wait) if dummy.sync_info else []
        for i in range(len(waits)):
            w = nc.sync.nop(nofuse=True, hint="tail_drain_wait")
            w.ins.sync_info = mybir.SyncInfo(on_wait=waits[i : i + 1], on_update=[])
        nc.sync.drain()
        nc.all_engine_barrier()
        assert self.sems is not None
        popped = nc._tile_sem_poison_stack.pop()
        assert popped is self._sem_poison
        nc.clear_and_free_semaphores(list(self.sems.allocated().values()))
        nc.all_engine_barrier()

    tile_mod.TileContext._drain_and_barrier = _drain_and_barrier

    # Body instructions can also accumulate >2 waits (CTRL structs take 1,
    # other structs 2 on this walrus).  Before lowering, move excess waits
    # onto single-wait nops inserted just before the instruction on the same
    # engine stream.
    _orig_lower = tile_mod.TileContext._lower_ordered_insts

    def _lower_split_waits(self, ordered):
        nc = self.nc
        for bb_name, insts in ordered.items():
            new_insts = []
            for inst in insts:
                si = getattr(inst, "sync_info", None)
                waits = list(si.on_wait) if si is not None and si.on_wait else []
                limit = 1
                if len(waits) > limit and inst.engine is not None:
                    keep = waits[: limit - 1] if limit > 1 else []
                    spill = waits[len(keep) :][:-1]
                    keep = keep + [waits[-1]]
                    for w in spill:
                        nop = mybir.InstNoOp(
                            name=nc.get_next_instruction_name(),
                            engine=inst.engine,
                            ins=[],
                            outs=[],
                        )
                        nop.sync_info = mybir.SyncInfo(on_wait=[w], on_update=[])
                        nop.debug = inst.debug
                        new_insts.append(nop)
                    inst.sync_info = mybir.SyncInfo(
                        on_wait=keep, on_update=list(si.on_update or [])
                    )
                new_insts.append(inst)
            ordered[bb_name] = new_insts
        return _orig_lower(self, ordered)

    tile_mod.TileContext._lower_ordered_insts = _lower_split_waits
    tile_mod.TileContext._drain_patched = True


def _build():
    import concourse.bass as bass
    import concourse.mybir as mybir
    import concourse.tile as tile
    from concourse.tile import add_dep_helper
    from concourse.masks import make_identity
    import concourse.bass as _bass

    _patch_tile_drain()
    f32 = mybir.dt.float32
    bf16 = mybir.dt.bfloat16

    nc = bass.Bass("TRN2", target_bir_lowering=False, debug=False, num_devices=NCORES)

    # ---- DRAM I/O (per core) ----
    xT = nc.dram_tensor("xT", [D, R], bf16, kind="ExternalInput").ap()
    # host-pretiled: [p, k*HDIM + c] = W.T[(k*128 + p), c] for this core's slice
    wqT = nc.dram_tensor("wqT", [128, NKT * HDIM], bf16, kind="ExternalInput").ap()
    wkT = nc.dram_tensor("wkT", [128, NKT * HDIM], bf16, kind="ExternalInput").ap()
    wvT = nc.dram_tensor("wvT", [128, NKT * HDIM], bf16, kind="ExternalInput").ap()
    bq_s = nc.dram_tensor("bq_s", [HDIM, 1], f32, kind="ExternalInput").ap()
    bk_s = nc.dram_tensor("bk_s", [HDIM, 1], f32, kind="ExternalInput").ap()
    bv_s = nc.dram_tensor("bv_s", [HDIM, 1], f32, kind="ExternalInput").ap()
    # core's 128-row slice of Wo^T: rows = this core's attention dims
    woT_c = nc.dram_tensor("woT_c", [HDIM, D], bf16, kind="ExternalInput").ap()
    masks_d = nc.dram_tensor("masks", [128, 4 * 1024], bf16, kind="ExternalInput").ap()
    # per-core output: quarter q rows land at [128q, 128(q+1)); host reassembles
    out = nc.dram_tensor("out", [NQ * 128, D], bf16, kind="ExternalOutput").ap()

    # ReduceScatter buffers per quarter: partial rows (quarter-local) in, 128
    # summed rows out
    cc_in = [nc.dram_tensor(f"cc_in{q}", [1024, D], bf16).ap() for q in range(NQ)]
    rs_out = [nc.dram_tensor(f"rs_out{q}", [128, D], bf16).ap() for q in range(NQ)]
    # first-strip softmax reciprocals bounce: rows 2j (head A), 2j+1 (head B)
    rec_d = nc.dram_tensor("rec_d", [2 * B * NSTRIP, 512], bf16).ap()

    wq = {"v": wvT, "q": wqT, "k": wkT}
    wb = {"v": bv_s, "q": bq_s, "k": bk_s}
    scale = 1.0 / float(np.sqrt(HD))

    with tile.TileContext(nc) as tc:
        import contextlib

        with contextlib.ExitStack() as ctx:
            singles = ctx.enter_context(tc.tile_pool(name="singles", bufs=1))

            # ---- on-chip constants first (Pool/DVE/Act engine work at t=0,
            # before any SWDGE descriptor generation queues up on Pool) ----
            ident = singles.tile([128, 128], bf16, tag="ident")
            make_identity(nc, ident)
            ones_row = singles.tile([1, 128], bf16, tag="ones")
            nc.vector.memset(ones_row, 1.0)
            # preload the Exp activation table off the critical path
            exp_warm = singles.tile([1, 128], bf16, tag="expwarm")
            nc.scalar.activation(
                out=exp_warm,
                in_=ones_row,
                func=mybir.ActivationFunctionType.Exp,
                scale=1.0,
            )

            # ---- weight/x DMAs.  SP queue (HWDGE): wv, x even-k chunks of
            # groups 0/1, wq, wk, masks, remaining even-k chunks.  Pool queue
            # (SWDGE; descriptor gen runs on the Pool engine, in parallel with
            # HWDGE): odd-k chunks, wo.  Biases ride the idle Act queue.
            # One SBUF tile per x chunk so QKV matmuls wait only on the exact
            # chunk they read.
            w_sb = {}
            bias_sb = {}
            for name in ("v", "q", "k"):
                w_sb[name] = singles.tile(
                    [128, NKT, HDIM], bf16, tag=f"w{name}", name=f"w{name}"
                )
                bias_sb[name] = singles.tile(
                    [HDIM, 1], f32, tag=f"b{name}", name=f"b{name}"
                )

            col_chunks = [(0, 512), (512, 1024), (1024, 2048), (2048, 3072), (3072, 4096)]
            xt_sb = [[None] * len(col_chunks) for _ in range(NKT)]

            def load_x_chunk(ci, k):
                lo, hi = col_chunks[ci]
                t = singles.tile(
                    [128, hi - lo], bf16, tag=f"xt{k}_{ci}", name=f"xt{k}_{ci}"
                )
                [nc.sync, nc.gpsimd][k % 2].dma_start(
                    out=t, in_=xT[128 * k : 128 * (k + 1), lo:hi]
                )
                xt_sb[k][ci] = t

            # weights arrive host-pretiled ([D, HDIM] -> [128, k, c] packed),
            # so each is one contiguous 2KB-per-partition DMA
            nc.sync.dma_start(
                out=w_sb["v"], in_=wq["v"].rearrange("p (k c) -> p k c", c=HDIM)
            )
            for k in range(0, NKT, 2):  # even k, chunk 0 (SP)
                load_x_chunk(0, k)
            for k in range(1, NKT, 2):  # odd k, chunk 0 (Pool)
                load_x_chunk(0, k)
            wo_sb = singles.tile([128, D], bf16, tag="wo")
            nc.gpsimd.dma_start(out=wo_sb, in_=woT_c)
            for name in ("q", "k"):
                nc.sync.dma_start(
                    out=w_sb[name],
                    in_=wq[name].rearrange("p (k c) -> p k c", c=HDIM),
                )
            for k in range(NKT):
                load_x_chunk(1, k)
            # biases ride late: their HWDGE slots must not delay the x chunks
            # that gate the first projections (first bias-add is at ~7us)
            for name in ("v", "q", "k"):
                nc.scalar.dma_start(out=bias_sb[name], in_=wb[name])
            mask_sb = singles.tile([128, 4 * 1024], bf16, tag="mask")
            nc.sync.dma_start(out=mask_sb, in_=masks_d)
            for ci in (2, 3, 4):
                for k in range(NKT):
                    load_x_chunk(ci, k)

            def xchunk(k, g):
                """x tile + column slice for 512-col group g of k-tile k."""
                ci, off = (g, 0) if g < 2 else (2 + (g - 2) // 2, 512 * ((g - 2) % 2))
                return xt_sb[k][ci][:, off : off + 512]

            # persistent activations
            qT_sb = singles.tile([128, R], bf16, tag="qT")   # rows 0-63 head A dims
            kT_sb = singles.tile([128, R], bf16, tag="kT")
            vT_sb = singles.tile([128, R], bf16, tag="vT")
            # v in [keys, dims] layout per key block kb:
            #   cols 0:64 = vA, 64 = ones, 65:129 = vB, 129 = ones
            # so lhsT for head h = cols [65h, 65h+65) = [v_h | ones]:
            # attnout at psum rows 0-63, softmax sums at row 64.
            v_ab = singles.tile([128, R // 128, 130], bf16, tag="vab")
            nc.vector.memset(v_ab[:, :, 64:65], 1.0)
            nc.vector.memset(v_ab[:, :, 129:130], 1.0)
            dests = {"v": vT_sb, "q": qT_sb, "k": kT_sb}

            # ---- PSUM rings (8 banks total):
            #   "sc"  2x[128,512]f32 (2 banks): scores (one tile per strip);
            #         borrowed for the second-strip reciprocal broadcast
            #   "fl"  2x[128,512]f32 (2 banks): woven QKV-group psums and V
            #         transposes (the PE "filler" stream)
            #   "av0" 2x[128,512]f32 (2 banks): strip-s0 psV pair, then s0's
            #         output-projection psums
            #   "av1" 2x[128,512]f32 (2 banks): same for strip s1
            with (
                tc.tile_pool(name="sc_ps", bufs=2, space="PSUM") as scp,
                tc.tile_pool(name="av_ps", bufs=2, space="PSUM") as avp,
                tc.tile_pool(name="p_sb", bufs=18) as ppool,
                tc.tile_pool(name="att_sb", bufs=6) as apool,
                tc.tile_pool(name="bc_sb", bufs=8) as bpool,
                tc.tile_pool(name="rec_sb", bufs=4) as rpool,
                tc.tile_pool(name="op_sb", bufs=8) as opool,
            ):
                av_tags = ["av0", "av1"]

                # QKV "filler" stream: each unit is a small closure (2 matmuls
                # of a projection, a bias add, or one V transpose) pulled
                # between attention key blocks so the PE fills the slack of
                # the exp-bound cadence with projection work.
                filler = []

                def push_qkv_group(g):
                    # q,k first: the next quarter's scores need only those;
                    # v (+transposes) is consumed one kb later by attnV
                    for name in ("q", "k", "v"):
                        holder = {}

                        def mm_pair(j, name=name, holder=holder, g=g):
                            def f():
                                if j == 0:
                                    holder["ps"] = avp.tile(
                                        [128, 512], f32, tag="fl",
                                        name=f"ps_{name}{g}",
                                    )
                                for k in (2 * j, 2 * j + 1):
                                    nc.tensor.matmul(
                                        holder["ps"],
                                        w_sb[name][:, k, :],
                                        xchunk(k, g),
                                        start=(k == 0),
                                        stop=(k == NKT - 1),
                                    )
                                if j == 3:
                                    nc.vector.tensor_scalar_add(
                                        dests[name][:, 512 * g : 512 * (g + 1)],
                                        holder["ps"],
                                        bias_sb[name],
                                    )
                            return f

                        for j in range(4):
                            filler.append(mm_pair(j))
                    for kb in range(4 * g, 4 * (g + 1)):
                        def tr(kb=kb):
                            tps = avp.tile([128, 128], bf16, tag="fl", name=f"vt{kb}")
                            nc.tensor.transpose(
                                tps, vT_sb[:, 128 * kb : 128 * (kb + 1)], ident
                            )
                            # strided copy: halves -> cols [0:64] and [65:129]
                            nc.vector.tensor_copy(
                                v_ab[:, kb, :].rearrange(
                                    "p (gg c) -> p gg c", c=65
                                )[:, :, 0:64],
                                tps.rearrange("p (gg c) -> p gg c", c=64),
                            )
                        filler.append(tr)

                def pull_filler(n):
                    for _ in range(n):
                        if filler:
                            filler.pop(0)()

                def emit_qkv_group_fast(g, tags=("fl", "av0", "av1")):
                    """Upfront group emission using three rings in parallel."""
                    for name, tag in zip(("q", "k", "v"), tags):
                        ps = avp.tile([128, 512], f32, tag=tag, name=f"ps_{name}{g}")
                        for k in range(NKT):
                            nc.tensor.matmul(
                                ps,
                                w_sb[name][:, k, :],
                                xchunk(k, g),
                                start=(k == 0),
                                stop=(k == NKT - 1),
                            )
                        nc.vector.tensor_scalar_add(
                            dests[name][:, 512 * g : 512 * (g + 1)],
                            ps,
                            bias_sb[name],
                        )
                    for kb in range(4 * g, 4 * (g + 1)):
                        tps = avp.tile([128, 128], bf16, tag=tags[0], name=f"vt{kb}")
                        nc.tensor.transpose(
                            tps, vT_sb[:, 128 * kb : 128 * (kb + 1)], ident
                        )
                        nc.vector.tensor_copy(
                            v_ab[:, kb, :].rearrange(
                                "p (gg c) -> p gg c", c=65
                            )[:, :, 0:64],
                            tps.rearrange("p (gg c) -> p gg c", c=64),
                        )

                def emit_outproj(att, quarter, si, cc_writes, engs, fill=0):
                    """Output projection for one 512-row strip (4 blocks of
                    128 rows); att is the normalized [128 dims, 512 rows]
                    lhsT.  Partials go quarter-local into cc_in[quarter].
                    Copy engines must be DVE/ACT (GPSIMD cannot read PSUM)."""
                    cp_engs, wr_engs = engs
                    split_wr = False
                    for r in range(4):
                        cop = opool.tile([128, D], bf16, tag="op")
                        for n in range(2):
                            ps = avp.tile([128, 512], f32, tag=av_tags[si])
                            nc.tensor.matmul(
                                ps,
                                att[:, 128 * r : 128 * (r + 1)],
                                wo_sb[:, 512 * n : 512 * (n + 1)],
                                start=True,
                                stop=True,
                            )
                            eng = cp_engs[(2 * r + n) % len(cp_engs)]
                            if eng is nc.scalar:
                                eng.copy(cop[:, 512 * n : 512 * (n + 1)], ps)
                            else:
                                eng.tensor_copy(cop[:, 512 * n : 512 * (n + 1)], ps)
                        pull_filler(fill)
                        rows = slice(512 * si + 128 * r, 512 * si + 128 * (r + 1))
                        if split_wr:
                            # tail-critical strip: two half-width writes land
                            # in parallel on separate queues
                            for n, eng in enumerate(wr_engs):
                                wr = eng.dma_start(
                                    out=cc_in[quarter][rows, 512 * n : 512 * (n + 1)],
                                    in_=cop[:, 512 * n : 512 * (n + 1)],
                                )
                                cc_writes.append(wr)
                        else:
                            wr = wr_engs[0].dma_start(
                                out=cc_in[quarter][rows, :], in_=cop
                            )
                            cc_writes.append(wr)

                rs_ccs = []

                def emit_quarter(b, s0, boundary_groups):
                    s1 = s0 + 1
                    quarter = 2 * b + s0 // 2
                    j0 = b * NSTRIP + s0
                    last_pair = quarter == NQ - 1
                    qc1 = slice(T * b + 512 * s1, T * b + 512 * (s1 + 1))
                    for g in boundary_groups:
                        push_qkv_group(g)
                    psV = {}
                    for sx in (s0, s1):
                        for h in ("A", "B"):
                            psV[(h, sx)] = avp.tile(
                                [128, 512],
                                f32,
                                tag=av_tags[sx - s0],
                                name=f"psV_{h}{b}{sx}",
                            )
                    nkb0, nkb1 = 4 * (s0 + 1), 4 * (s1 + 1)
                    cc_writes = []

                    def emit_attnv(kb, p_of):
                        # attnV accumulate; lhsT = [v_h | ones]: attnout rows
                        # 0-63, softmax sums row 64.  Runs one kb behind the
                        # scores so the PE never waits on the exp it just fed.
                        gkb = (T // 128) * b + kb
                        both = kb < nkb0
                        m = kb - 4 * (s0 if both else s1)
                        off = 128 * m if m >= 0 else 0
                        for hi, h in enumerate(("A", "B")):
                            p0, p1 = p_of[h]
                            lhsT = v_ab[:, gkb, 65 * hi : 65 * hi + 65]
                            if both:
                                nc.tensor.matmul(
                                    psV[(h, s0)][0:65, off:512],
                                    lhsT,
                                    p0[:, off:512],
                                    start=(kb == 0),
                                    stop=(kb == nkb0 - 1),
                                )
                                nc.tensor.matmul(
                                    psV[(h, s1)][0:65, 0:512],
                                    lhsT,
                                    p1,
                                    start=(kb == 0),
                                    stop=(kb == nkb1 - 1),
                                )
                            else:
                                nc.tensor.matmul(
                                    psV[(h, s1)][0:65, off:512],
                                    lhsT,
                                    p1[:, off:512],
                                    start=False,
                                    stop=(kb == nkb1 - 1),
                                )

                    def emit_norm_s0():
                        # strip s0: reciprocal -> DRAM bounce -> partition-
                        # broadcast DMA -> normalize.  Latency hidden behind
                        # strip s1's remaining key blocks.
                        rec = rpool.tile([128, 1024], bf16, tag="rec")
                        with nc.allow_low_precision("bf16 softmax recip"):
                            nc.vector.reciprocal(
                                rec[64:65, 0:512], psV[("A", s0)][64:65, :]
                            )
                            nc.vector.reciprocal(
                                rec[64:65, 512:1024], psV[("B", s0)][64:65, :]
                            )
                        nc.gpsimd.dma_start(
                            out=rec_d[2 * j0 : 2 * j0 + 2, :], in_=rec[64:65, :]
                        )
                        att = apool.tile([128, 512], bf16, tag="att")
                        for hi, h in enumerate(("A", "B")):
                            r_ap = rec_d[2 * j0 + hi : 2 * j0 + hi + 1, :]
                            bc_ap = _bass.AP(
                                tensor=r_ap.tensor,
                                offset=r_ap.offset,
                                ap=[[0, 64]] + list(r_ap.ap[1:]),
                            )
                            bc = bpool.tile([64, 512], bf16, tag="bc")
                            nc.gpsimd.dma_start(out=bc, in_=bc_ap)
                            nc.vector.tensor_mul(
                                att[64 * hi : 64 * (hi + 1), :],
                                psV[(h, s0)][0:64, :],
                                bc,
                            )
                        return att

                    def emit_norm_s1():
                        # strip s1 is on the critical path to the quarter's
                        # ReduceScatter: broadcast reciprocals via a
                        # [1,64]-ones matmul into a borrowed scores slot plus
                        # one PSUM->SBUF copy — no DRAM round trip.
                        rec = rpool.tile([1, 1024], bf16, tag="rec")
                        with nc.allow_low_precision("bf16 softmax recip"):
                            nc.vector.reciprocal(
                                rec[0:1, 0:512], psV[("A", s1)][64:65, :]
                            )
                            nc.vector.reciprocal(
                                rec[0:1, 512:1024], psV[("B", s1)][64:65, :]
                            )
                        # bcp lives in the av0 ring (free after s0's outproj):
                        # borrowing the scores ring would couple this
                        # quarter's normalize into the next quarter's scores
                        bcp = avp.tile([128, 512], f32, tag="av0", name="bc_ps")
                        nc.tensor.matmul(
                            bcp[0:64, :],
                            ones_row[0:1, 0:64],
                            rec[0:1, 0:512],
                            start=True,
                            stop=True,
                        )
                        nc.tensor.matmul(
                            bcp[64:128, :],
                            ones_row[0:1, 0:64],
                            rec[0:1, 512:1024],
                            start=True,
                            stop=True,
                        )
                        bcs = bpool.tile([128, 512], bf16, tag="bcs", bufs=2)
                        nc.scalar.copy(bcs, bcp)
                        att = apool.tile([128, 512], bf16, tag="att")
                        nc.vector.tensor_mul(
                            att[0:64, :], psV[("A", s1)][0:64, :], bcs[0:64, :]
                        )
                        nc.vector.tensor_mul(
                            att[64:128, :], psV[("B", s1)][0:64, :], bcs[64:128, :]
                        )
                        return att

                    prev = None
                    for kb in range(nkb1):
                        krange = slice(T * b + 128 * kb, T * b + 128 * (kb + 1))
                        both = kb < nkb0
                        p_of = {}
                        # scores + exp + mask for BOTH heads and both strips;
                        # separate [128,512] psum tiles per strip keep the
                        # scores ring at 2 banks so the QKV filler gets PSUM
                        for hi, h in enumerate(("A", "B")):
                            rows = slice(64 * hi, 64 * (hi + 1))
                            p0 = p1 = None
                            if both:
                                m = kb - 4 * s0
                                off = 128 * m if m >= 0 else 0
                                psS0 = scp.tile([128, 512], f32, tag="sc")
                                p0 = ppool.tile([128, 512], bf16, tag="p")
                                nc.tensor.matmul(
                                    psS0[:, off:512],
                                    kT_sb[rows, krange],
                                    qT_sb[
                                        rows,
                                        T * b + 512 * s0 + off
                                        : T * b + 512 * (s0 + 1),
                                    ],
                                    start=True,
                                    stop=True,
                                    tile_position=(64 * hi, 0),
                                )
                                psS1 = scp.tile([128, 512], f32, tag="sc")
                                p1 = ppool.tile([128, 512], bf16, tag="p")
                                nc.tensor.matmul(
                                    psS1,
                                    kT_sb[rows, krange],
                                    qT_sb[rows, qc1],
                                    start=True,
                                    stop=True,
                                    tile_position=(64 * hi, 0),
                                )
                                nc.scalar.activation(
                                    out=p0[:, off:512],
                                    in_=psS0[:, off:512],
                                    func=mybir.ActivationFunctionType.Exp,
                                    scale=scale,
                                )
                                if m >= 0:
                                    # mask the diagonal block: DVE 4x mode
                                    # (~200ns) vs GPSIMD's 0.42-efficiency
                                    # software multiply (~1us)
                                    nc.vector.tensor_mul(
                                        p0[:, off:512],
                                        p0[:, off:512],
                                        mask_sb[:, 1024 * m + off : 1024 * m + 512],
                                    )
                                nc.scalar.activation(
                                    out=p1,
                                    in_=psS1,
                                    func=mybir.ActivationFunctionType.Exp,
                                    scale=scale,
                                )
                            else:
                                m = kb - 4 * s1
                                off = 128 * m if m >= 0 else 0
                                psS1 = scp.tile([128, 512], f32, tag="sc")
                                p1 = ppool.tile([128, 512], bf16, tag="p")
                                nc.tensor.matmul(
                                    psS1[:, off:512],
                                    kT_sb[rows, krange],
                                    qT_sb[
                                        rows,
                                        T * b + 512 * s1 + off
                                        : T * b + 512 * (s1 + 1),
                                    ],
                                    start=True,
                                    stop=True,
                                    tile_position=(64 * hi, 0),
                                )
                                nc.scalar.activation(
                                    out=p1[:, off:512],
                                    in_=psS1[:, off:512],
                                    func=mybir.ActivationFunctionType.Exp,
                                    scale=scale,
                                )
                                if m >= 0:
                                    nc.vector.tensor_mul(
                                        p1[:, off:512],
                                        p1[:, off:512],
                                        mask_sb[:, 1024 * m + off : 1024 * m + 512],
                                    )
                            p_of[h] = (p0, p1)
                            if hi == 0 and prev is not None:
                                # weave the previous block's attnV between the
                                # two heads' scores: covers the exp latency
                                # that gates the scores-ring reuse
                                emit_attnv(*prev)
                                if prev[0] == nkb0 - 1:
                                    # strip s0 complete: normalize + project
                                    # it while s1's key blocks keep PE/ACT
                                    # busy.  Normal priority: hoisting this
                                    # chain would stall the remaining key
                                    # blocks' scores behind it on the
                                    # in-order PE stream.
                                    att0 = emit_norm_s0()
                                    emit_outproj(
                                        att0,
                                        quarter,
                                        0,
                                        cc_writes,
                                        ([nc.vector, nc.scalar]
                                         if last_pair else [nc.vector],
                                         [nc.sync]),
                                    )
                        prev = (kb, p_of)
                        pull_filler(-(-len(filler) // max(1, nkb1 - kb)))
                    emit_attnv(*prev)
                    # boundary: the second strip's normalize + projection +
                    # ReduceScatter; high priority only for the last quarter,
                    # whose chain sets the final ReduceScatter (and thus
                    # program end); earlier quarters have collective-queue
                    # slack and should yield to the next quarter's exp stream
                    with (tc.high_priority() if last_pair
                          else contextlib.nullcontext()):
                        att1 = emit_norm_s1()
                        # ACT is idle once the quarter's exps drain, so it
                        # takes half the boundary copies; DVE takes the rest
                        engs = (
                            ([nc.scalar, nc.vector], [nc.sync, nc.scalar])
                            if last_pair
                            else ([nc.vector, nc.scalar, nc.vector, nc.vector], [nc.sync])
                        )
                        emit_outproj(att1, quarter, 1, cc_writes, engs)
                        # fire this quarter's ReduceScatter (adds partials
                        # across cores; core j gets rows [128j, 128(j+1)))
                        cc = nc.gpsimd.collective_compute(
                            "ReduceScatter",
                            mybir.AluOpType.add,
                            ins=[cc_in[quarter]],
                            outs=[rs_out[quarter]],
                            replica_groups=[list(range(NCORES))],
                        )
                        for wr in cc_writes:
                            add_dep_helper(
                                cc.ins, wr.ins, sync=True, reason="cc in ready"
                            )
                    rs_ccs.append((quarter, cc))
                    # the hop for the quarter TWO collectives back is safe to
                    # emit here: its gate completed long ago, so it drains
                    # mid-stream instead of serializing into the program tail
                    if len(rs_ccs) >= 3:
                        pq, pcc = rs_ccs[len(rs_ccs) - 3]
                        mv = nc.sync.dma_start(
                            out=out[128 * pq : 128 * (pq + 1), :],
                            in_=rs_out[pq],
                        )
                        add_dep_helper(mv.ins, pcc.ins, sync=True, reason="rs done")
                    # drain any remaining woven QKV work before the next
                    # quarter's scores need its projections
                    pull_filler(len(filler))

                # warm the PE p-state with ~3us of dependency-free matmuls
                # so the QKV prefix runs at full speed from its first cycle
                warm = avp.tile([128, 512], f32, tag="fl", name="warm_ps")
                for i in range(24):
                    nc.tensor.matmul(
                        warm[:, 0:128], ident, ident, start=True, stop=True
                    )
                emit_qkv_group_fast(0)
                push_qkv_group(1)
                emit_quarter(0, 0, (2, 3))      # quarter 0: rows 0-1023
                emit_quarter(0, 2, (4, 5, 6))   # quarter 1: rows 1024-2047
                emit_quarter(1, 0, (7,))        # quarter 2: rows 2048-3071
                emit_quarter(1, 2, ())          # quarter 3: rows 3072-4095

                # final hops: DRAM->DRAM, gated on each quarter's collective.
                # tile_wait_until keeps the greedy scheduler from dispatching
                # these into idle queue slots mid-program, where their
                # collective-semaphore waits would head-of-line-block the
                # queue (observed: a d2d parked on SP stalled a whole quarter)
                # quarter NQ-2's hop: emitted after the last collective, so
                # its gate (two collectives back) is already satisfied
                pq, pcc = rs_ccs[NQ - 2]
                mv = nc.sync.dma_start(
                    out=out[128 * pq : 128 * (pq + 1), :], in_=rs_out[pq]
                )
                add_dep_helper(mv.ins, pcc.ins, sync=True, reason="rs done")
                with tc.tile_wait_until(1.0):
                    for quarter, cc in rs_ccs:
                        if quarter == NQ - 1:
                            # tail-critical: two parallel half hops
                            for half, eng in ((0, nc.sync), (1, nc.scalar)):
                                lo = 128 * quarter + 64 * half
                                mv = eng.dma_start(
                                    out=out[lo : lo + 64, :],
                                    in_=rs_out[quarter][64 * half : 64 * half + 64, :],
                                )
                                add_dep_helper(
                                    mv.ins, cc.ins, sync=True, reason="rs done"
                                )
                        else:
                            pass  # quarters 0..NQ-2 already hopped mid-stream

    return nc


def _host_prep(x, Wq, bq, Wk, bk, Wv, bv, Wo, bo):
    """Build the 8 per-core input maps."""
    x = np.asarray(x, np.float32)
    xT = np.ascontiguousarray(x.reshape(R, D).T).astype(_BF16)
    woT = np.ascontiguousarray(np.asarray(Wo, np.float32).T).astype(_BF16)

    # multiplicative causal masks for the 4 diagonal offsets; each 1024-col
    # block m is [mask_m | ones] so a merged strip-pair tile can be masked in
    # one op (second strip fully visible when the first is on the diagonal).
    masks = np.ones((128, 4 * 1024), np.float32)
    r = np.arange(128)[:, None]
    c = np.arange(512)[None, :]
    for m in range(4):
        masks[:, 1024 * m : 1024 * m + 512] = (r + 128 * m <= c).astype(np.float32)
    masks = masks.astype(_BF16)

    def tile_w(W, hs):
        # [D, HDIM] -> [128, NKT*HDIM] with [p, k*HDIM+c] = W.T[k*128+p, c]
        wT = np.asarray(W, np.float32)[hs, :].T.astype(_BF16)
        return np.ascontiguousarray(
            wT.reshape(NKT, 128, HDIM).transpose(1, 0, 2).reshape(128, NKT * HDIM)
        )

    in_maps = []
    for core in range(NCORES):
        hs = slice(HDIM * core, HDIM * (core + 1))
        in_maps.append(
            {
                "xT": xT,
                "wqT": tile_w(Wq, hs),
                "wkT": tile_w(Wk, hs),
                "wvT": tile_w(Wv, hs),
                "bq_s": np.asarray(bq, np.float32)[hs].reshape(HDIM, 1).copy(),
                "bk_s": np.asarray(bk, np.float32)[hs].reshape(HDIM, 1).copy(),
                "bv_s": np.asarray(bv, np.float32)[hs].reshape(HDIM, 1).copy(),
                "woT_c": np.ascontiguousarray(woT[hs, :]),
                "masks": masks,
            }
        )
    return in_maps


def _run(in_maps, trace=False):
    from concourse import bass_utils

    if "nc" not in _cache:
        _cache["nc"] = _build()
    nc = _cache["nc"]
    if trace:
        try:
            res = bass_utils.run_bass_kernel_spmd(
                nc, in_maps, core_ids=list(range(NCORES)), trace=True
            )
            return res
        except Exception:
            pass  # NTFF hook unavailable under this axon build
    try:
        res = bass_utils.run_bass_kernel_spmd(
            nc, in_maps, core_ids=list(range(NCORES)), trace=False
        )
    except Exception:
        # transient device faults (NRT_EXEC_UNIT_UNRECOVERABLE) clear on retry
        res = bass_utils.run_bass_kernel_spmd(
            nc, in_maps, core_ids=list(range(NCORES)), trace=False
        )
    return res


def kernel(x, Wq, bq, Wk, bk, Wv, bv, Wo, bo, _trace=False, _want_results=False):
    in_maps = _host_prep(x, Wq, bq, Wk, bk, Wv, bv, Wo, bo)
    res = _run(in_maps, trace=_trace)
    # core j's output rows: quarter q chunk = global rows 1024q + [128j, 128(j+1))
    full = np.zeros((R, D), np.float32)
    for j in range(NCORES):
        part = np.asarray(res.results[j]["out"], np.float32)
        for q in range(NQ):
            full[1024 * q + 128 * j : 1024 * q + 128 * (j + 1)] = part[
                128 * q : 128 * (q + 1)
            ]
    full += np.asarray(bo, np.float32)[None, :]
    full = full.reshape(B, T, D)
    if _want_results:
        return full, res
    return full
